# revision 31
# baseline (speedup 1.0000x reference)
"""GAT (2-layer, 8-head) fused Bass kernel for 8 trn2 NeuronCores.

Sharding: nodes (rows of x) split 512/core. Per core: h computed key-major
with fused score columns; h (bf16) + s_dst (fp32) AllGather'd; each core
computes its 512xN attention block for all 8 heads; layer-1 output projected
and AllGather'd (18 fp32 cols); each core computes its 512xN layer-2 block
and the final log_softmax rows.

Key algebra: with s_i = h_i . a_src, d_j = h_j . a_dst,
  exp(leakyrelu(s_i + d_j)) = max(exp(s_i)exp(d_j), exp(.2 s_i)exp(.2 d_j))
and softmax over j is invariant to any per-i scale, so the attention
numerator is P[j,i] = max(b_j, w_i * dd_j) with b_j = exp(d_j),
w_i = exp(-0.8 s_i), dd_j = exp(0.2 d_j).

P tiles [128 keys, 512 queries] are produced on three engines:
  DVE/Pool: tensor_scalar (mult, max) -> P
  ACT:      relu(dd_j * w_i - b_j) = P - b_j, single activation op; the
            missing rank-1 term hb[c] = sum_j b_j hx[j,c] over ACT-tiles is
            added back into the PSUM accumulation via two tiny matmuls.
Attention matmuls run with the P chunk [128k x 128q] as the *stationary*
operand and the per-head hx block [128, 65] (64 h cols + ones) as the
*moving* operand: 65 columns/matmul instead of 512 -> ~2x less PE time,
and the output lands query-major so normalize/elu/log_softmax use cheap
per-partition scalars.
"""

import numpy as np

N, NFEAT, NHID, NCLASS, NHEADS = 4096, 512, 64, 16, 8
NC = 8                      # cores
NQ = N // NC                # 512 own nodes per core
QT = NQ // 128              # 4 query tiles per core
JT = N // 128               # 32 key tiles
NCH = JT // NC              # 4 key tiles per AG chunk
ALPHA = 0.2
HW = NHID * NHEADS          # 512
HXC = NHEADS * (NHID + 1)   # 520: per-head 64 h cols + ones col
AGC2 = 18                   # AG2: 16 outh + 1 ones + 1 sdst2

# engine schedule for the 32 P-tiles of each layer-1 head sweep
_ACT1 = (7, 14, 21, 28)
_SCHED1 = ['D'] * JT
for _p in _ACT1:
    _SCHED1[_p] = 'A'
for _p in (2, 9, 16, 23, 30):
    _SCHED1[_p] = 'P'
# layer-2: 32 tiles
_SCHED2 = ['D'] * JT
for _p in (3, 11, 19, 27):
    _SCHED2[_p] = 'A'
for _p in (6, 14, 22, 29):
    _SCHED2[_p] = 'P'

_CACHE = {}


def _build_nc(no_cc=False, no_l1=False):
    import concourse.bass as bass
    import concourse.bacc as bacc
    import concourse.mybir as mybir
    import concourse.tile as tile
    from concourse.masks import make_identity

    fp32 = mybir.dt.float32
    bf16 = mybir.dt.bfloat16
    AX = mybir.AxisListType.X
    OP = mybir.AluOpType
    AF = mybir.ActivationFunctionType

    nc = bacc.Bacc()
    xT = nc.declare_dram_parameter("xT", [NFEAT, NQ], bf16, isOutput=False)
    Whr = nc.declare_dram_parameter("Whr", [NFEAT, HW], bf16, isOutput=False)
    WhrT = nc.declare_dram_parameter("WhrT", [HW, NFEAT], bf16, isOutput=False)
    Asd = nc.declare_dram_parameter("Asd", [HW, 16], bf16, isOutput=False)
    Wo = nc.declare_dram_parameter("Wo", [HW, NCLASS], bf16, isOutput=False)
    aod = nc.declare_dram_parameter("aod", [2, NCLASS], fp32, isOutput=False)
    out = nc.declare_dram_parameter("out", [NQ, NCLASS], fp32, isOutput=True)

    with tile.TileContext(nc) as tc:
        with (
            tc.tile_pool(name="const", bufs=1) as constp,
            tc.tile_pool(name="big", bufs=1) as bigp,
            tc.tile_pool(name="work", bufs=3) as workp,
            tc.tile_pool(name="pp", bufs=44) as ppool,
            tc.tile_pool(name="ps_acc", bufs=3, space="PSUM") as ps_acc,
            tc.tile_pool(name="ps_t", bufs=3, space="PSUM") as ps_t,
            tc.tile_pool(name="ps_hb", bufs=2, space="PSUM") as ps_hb,
            tc.tile_pool(name="dram", bufs=1, space="DRAM") as dramp,
        ):
            v, sc, g, te, dma = nc.vector, nc.scalar, nc.gpsimd, nc.tensor, nc.sync

            ident = constp.tile([128, 128], fp32, tag="ident")
            make_identity(nc, ident[:])
            ident_bf = constp.tile([128, 128], bf16, tag="ident_bf")
            v.tensor_copy(ident_bf[:], ident[:])
            ones1 = constp.tile([1, 128], bf16, tag="ones1")
            g.memset(ones1[:], 1.0)
            # sel[k, h*128+m] = 1 iff k == h (partition-broadcast matmuls)
            self_f = constp.tile([8, 8 * 128], fp32, tag="self_f")
            g.memset(self_f[:], 0.0)
            g.affine_select(
                out=self_f[:].rearrange("k (h m) -> k h m", m=128),
                in_=self_f[:].rearrange("k (h m) -> k h m", m=128),
                compare_op=mybir.AluOpType.not_equal,
                fill=1.0, base=0, channel_multiplier=1,
                pattern=[[-1, 8], [0, 128]])
            sel_bf = constp.tile([8, 8 * 128], bf16, tag="sel_bf")
            sc.copy(sel_bf[:], self_f[:])

            # ---- A. param loads (WhrT/Asd first: the score chain gates the
            # attention panels, which gate all P-tile production) ----
            whrT_sb = constp.tile([128, 4, NFEAT], bf16, tag="whrT_sb")
            dma.dma_start(whrT_sb[:], WhrT.rearrange("(k p) f -> p k f", p=128))
            asd_sb = constp.tile([128, 4, 16], bf16, tag="asd_sb")
            dma.dma_start(asd_sb[:], Asd.rearrange("(k p) s -> p k s", p=128))
            xT_sb = constp.tile([128, 4, NQ], bf16, tag="xT_sb")
            dma.dma_start(xT_sb[:], xT.rearrange("(k p) q -> p k q", p=128))
            whr_sb = constp.tile([128, 4, HW], bf16, tag="whr_sb")
            dma.dma_start(whr_sb[:], Whr.rearrange("(k p) c -> p k c", p=128))
            wo_sb = constp.tile([128, 4, 16], bf16, tag="wo_sb")
            dma.dma_start(wo_sb[:], Wo.rearrange("(k p) s -> p k s", p=128))
            aos_b = constp.tile([128, 16], fp32, tag="aos_b")
            dma.dma_start(aos_b[:], aod[0:1, :].to_broadcast((128, 16)))
            aod_b = constp.tile([128, 16], fp32, tag="aod_b")
            dma.dma_start(aod_b[:], aod[1:2, :].to_broadcast((128, 16)))

            ag1h_in = dramp.tile([NQ, HXC], bf16, tag="ag1h_in")
            ag1h_out = dramp.tile([N, HXC], bf16, tag="ag1h_out",
                                  addr_space="Local" if no_cc else "Shared")
            ag1s_in = dramp.tile([NQ, 8], fp32, tag="ag1s_in")
            ag1s_out = dramp.tile([N, 8], fp32, tag="ag1s_out",
                                  addr_space="Local" if no_cc else "Shared")
            ag2_in = dramp.tile([NQ, AGC2], fp32, tag="ag2_in")
            ag2_out = dramp.tile([N, AGC2], fp32, tag="ag2_out",
                                 addr_space="Local" if no_cc else "Shared")

            # ---- C. Wa_feat = Whr @ Asd; s_own; stage + AllGather s ----
            wa_ps = ps_t.tile([16, NFEAT], fp32, tag="tp", name="wa_ps")
            for k in range(4):
                te.matmul(wa_ps[:], asd_sb[:, k, :], whrT_sb[:, k, :],
                          start=(k == 0), stop=(k == 3))
            waT_sb = constp.tile([16, NFEAT], bf16, tag="waT_sb")
            sc.copy(waT_sb[:], wa_ps[:])
            waf_ps = ps_t.tile([128, 4, 16], bf16, tag="tp", name="waf_ps")
            for k in range(4):
                te.transpose(waf_ps[:, k, :], waT_sb[:, k * 128:(k + 1) * 128],
                             ident_bf[0:16, 0:16])
            wa_f = constp.tile([128, 4, 16], bf16, tag="wa_f")
            v.tensor_copy(wa_f[:], waf_ps[:])

            stgs_all = bigp.tile([128, QT, 16], fp32, tag="stgs_all")
            for qt in range(QT):
                s_ps = ps_t.tile([128, 16], fp32, tag="tp", name="s_ps")
                for k in range(4):
                    te.matmul(s_ps[:], xT_sb[:, k, qt * 128:(qt + 1) * 128],
                              wa_f[:, k, :], start=(k == 0), stop=(k == 3))
                v.tensor_copy(stgs_all[:, qt, :], s_ps[:])
            dma.dma_start(ag1s_in.rearrange("(a p) h -> p a h", p=128),
                          stgs_all[:, :, 8:16])
            if no_cc:
                dma.dma_start(
                    ag1s_out.rearrange("(r q) h -> r q h", r=NC),
                    ag1s_in.rearrange("(o q) h -> o q h", o=1).to_broadcast(
                        (NC, NQ, 8)))
            else:
                g.collective_compute(
                    "AllGather", OP.bypass,
                    ins=[ag1s_in.opt()], outs=[ag1s_out.opt()],
                    replica_groups=[list(range(NC))],
                )
            sd_pan = constp.tile([128, JT * 8], fp32, tag="sd_pan")
            dma.dma_start(
                sd_pan[:].rearrange("p (t h) -> p t h", h=8),
                ag1s_out.rearrange("(t p) h -> p t h", p=128))

            # ---- D. w panel (own s_src): transpose, exp, broadcast ----
            s_fm = ps_t.tile([16, NQ], fp32, tag="tp", name="s_fm")
            for qt in range(QT):
                te.transpose(s_fm[:, qt * 128:(qt + 1) * 128],
                             stgs_all[:, qt, :], ident[0:128, 0:128])
            w_bf = constp.tile([8, NQ], bf16, tag="w_bf")
            sc.activation(w_bf[:], s_fm[0:8, :], AF.Exp, scale=-0.8)
            wb_all = constp.tile([128, NHEADS, NQ], bf16, tag="wb_all")
            for h in range(NHEADS):
                wb_ps = ps_t.tile([128, NQ], fp32, tag="tp", name="wb_ps")
                te.matmul(wb_ps[:], sel_bf[:, h * 128:(h + 1) * 128], w_bf[:],
                          start=True, stop=True)
                sc.copy(wb_all[:, h, :], wb_ps[:])

            # ---- B. h_own key-major; stage + AllGather h ----
            stg = bigp.tile([128, QT, HXC], bf16, tag="stg")
            for qt in range(QT):
                h_ps = ps_acc.tile([128, HW], fp32, tag="acc", name="h_ps")
                for k in range(4):
                    te.matmul(h_ps[:], xT_sb[:, k, qt * 128:(qt + 1) * 128],
                              whr_sb[:, k, :], start=(k == 0), stop=(k == 3))
                eng = sc if qt % 2 else v
                eng_c = eng.copy if qt % 2 else eng.tensor_copy
                eng_c(stg[:, qt, :].rearrange("p (h c) -> p h c", c=65)[:, :, 0:64],
                      h_ps[:].rearrange("p (h c) -> p h c", c=64))
                g.memset(
                    stg[:, qt, :].rearrange("p (h c) -> p h c", c=65)[:, :, 64:65],
                    1.0)
            dma.dma_start(ag1h_in.rearrange("(a p) x -> p a x", p=128), stg[:])

            # ---- E. AG1h replica copies interleaved with hx chunk loads ----
            hx = []
            for c in range(NC):
                if no_cc:
                    dma.dma_start(ag1h_out[c * NQ:(c + 1) * NQ, :],
                                  ag1h_in[:])
                elif c == 0:
                    g.collective_compute(
                        "AllGather", OP.bypass,
                        ins=[ag1h_in.opt()], outs=[ag1h_out.opt()],
                        replica_groups=[list(range(NC))],
                    )
                t = bigp.tile([128, NCH, HXC], bf16, tag=f"hx{c}",
                              name=f"hx{c}")
                dma.dma_start(
                    t[:],
                    ag1h_out[c * NQ:(c + 1) * NQ, :].rearrange(
                        "(a p) x -> p a x", p=128))
                hx.append(t)

            # ---- F. key-side score panels ----
            b_all = constp.tile([128, JT * 8], fp32, tag="b_all")
            sc.activation(b_all[:], sd_pan[:], AF.Exp)
            d_all = constp.tile([128, JT * 8], fp32, tag="d_all")
            sc.activation(d_all[:], sd_pan[:], AF.Exp, scale=ALPHA)
            nb_all = constp.tile([128, JT * 8], fp32, tag="nb_all")
            v.tensor_scalar(nb_all[:], b_all[:], -1.0, None, OP.mult)
            b_bf = constp.tile([128, JT * 8], bf16, tag="b_bf")
            v.tensor_copy(b_bf[:], b_all[:])

            def hx_slice(jt, h, w):
                return hx[jt // NCH][:, jt % NCH, h * 65:h * 65 + w]

            # ---- H. layer-1 attention ----
            xc_pre = [bigp.tile([128, HW], fp32, tag=f"xc{qc}",
                                name=f"xc{qc}") for qc in range(QT)]
            r_pan = constp.tile([128, NHEADS, QT], fp32, tag="r_pan")

            for h in range(NHEADS if not no_l1 else 0):
                acc = ps_acc.tile([128, QT, 65], fp32, tag="acc", name="acc")
                act_jts = []
                hb_ps = ps_hb.tile([1, 65], fp32, tag="hb", name="hb_ps")
                for idx in range(JT):
                    jt = (h * NCH + idx) % JT
                    col = slice(jt * 8 + h, jt * 8 + h + 1)
                    e = _SCHED1[idx]
                    pt = ppool.tile([128, NQ], bf16, tag="pt", name="pt")
                    if e == 'A':
                        sc.activation(pt[:], wb_all[:, h, :], AF.Relu,
                                      bias=nb_all[:, col], scale=d_all[:, col])
                        te.matmul(hb_ps[:], b_bf[:, col], hx_slice(jt, h, 65),
                                  start=(not act_jts), stop=(idx == _ACT1[-1]))
                        act_jts.append(jt)
                    else:
                        eng = v if e == 'D' else g
                        eng.tensor_scalar(pt[:], wb_all[:, h, :],
                                          d_all[:, col], b_all[:, col],
                                          OP.mult, OP.max)
                    for qc in range(QT):
                        te.matmul(acc[:, qc, :],
                                  pt[:, qc * 128:(qc + 1) * 128],
                                  hx_slice(jt, h, 65),
                                  start=(idx == 0), stop=False)
                # rank-1 correction for the ACT-produced tiles: acc += 1 (x) hb
                hb_sb = workp.tile([1, 65], bf16, tag="hb_sb", bufs=2)
                v.tensor_copy(hb_sb[:], hb_ps[:])
                for qc in range(QT):
                    te.matmul(acc[:, qc, :], ones1[:], hb_sb[:],
                              start=False, stop=True)
                # normalize: r = 1/den, xc_pre[:, h*64:] = f * r
                v.reciprocal(r_pan[:, h, :], acc[:, :, 64])
                for qc in range(QT):
                    sc.activation(xc_pre[qc][:, h * 64:(h + 1) * 64],
                                  acc[:, qc, 0:64], AF.Copy,
                                  scale=r_pan[:, h, qc:qc + 1])

            if no_l1:
                for qc in range(QT):
                    g.memset(xc_pre[qc][:], 0.5)

            # ---- I/J. per-query-tile: elu (fp32), transpose, project,
            # stage AG2 ----
            w2tmp = constp.tile([128, QT], fp32, tag="w2tmp")
            stg2 = bigp.tile([128, QT, AGC2], fp32, tag="stg2")
            for qc in range(QT):
                e1 = workp.tile([128, HW], fp32, tag="elu_e", name="e1")
                sc.activation(e1[:], xc_pre[qc][:], AF.Exp)
                g.tensor_scalar(e1[:], e1[:], 1.0, 0.0, OP.subtract, OP.min)
                v.tensor_tensor(xc_pre[qc][:], xc_pre[qc][:], e1[:], OP.max)
                tp = ps_t.tile([128, 4, 128], fp32, tag="tp", name="tp_xc")
                for fc in range(4):
                    te.transpose(tp[:, fc, :],
                                 xc_pre[qc][:, fc * 128:(fc + 1) * 128],
                                 ident[:])
                xcT = bigp.tile([128, 4, 128], bf16, tag=f"xcT{qc}",
                                name=f"xcT{qc}")
                sc.copy(xcT[:], tp[:])
                o_ps = ps_t.tile([128, 16], fp32, tag="tp", name="o_ps")
                for fc in range(4):
                    te.matmul(o_ps[:], xcT[:, fc, :], wo_sb[:, fc, :],
                              start=(fc == 0), stop=(fc == 3))
                v.tensor_copy(stg2[:, qc, 0:16], o_ps[:])
                g.memset(stg2[:, qc, 16:17], 1.0)
                tmp = workp.tile([128, 16], fp32, tag="sdtmp")
                v.scalar_tensor_tensor(tmp[:], o_ps[:], 1.0, aod_b[:],
                                       OP.mult, OP.mult,
                                       accum_out=stg2[:, qc, 17:18])
                tmp2 = workp.tile([128, 16], fp32, tag="sdtmp2")
                v.scalar_tensor_tensor(tmp2[:], o_ps[:], 1.0, aos_b[:],
                                       OP.mult, OP.mult,
                                       accum_out=w2tmp[:, qc:qc + 1])
            dma.dma_start(ag2_in.rearrange("(a p) c -> p a c", p=128), stg2[:])

            # ---- K. w2 broadcast panel ----
            w2e = constp.tile([128, QT], fp32, tag="w2e")
            sc.activation(w2e[:], w2tmp[:], AF.Exp, scale=-0.8)
            w2tp = ps_hb.tile([QT, 128], fp32, tag="hb", name="w2tp")
            te.transpose(w2tp[:], w2e[:], ident[:])
            w2T_bf = constp.tile([QT, 128], bf16, tag="w2T_bf")
            v.tensor_copy(w2T_bf[:], w2tp[:])
            w2b_ps = ps_t.tile([128, QT, 128], fp32, tag="tp", name="w2b_ps")
            for qt in range(QT):
                te.matmul(w2b_ps[:, qt, :],
                          sel_bf[0:QT, qt * 128:(qt + 1) * 128], w2T_bf[:],
                          start=True, stop=True)
            w2b = constp.tile([128, NQ], bf16, tag="w2b")
            sc.copy(w2b[:], w2b_ps[:].rearrange("p a q -> p (a q)"))

            # ---- L. AllGather 2 + panels (two halves for earlier start) ----
            if no_cc:
                dma.dma_start(
                    ag2_out.rearrange("(r q) c -> r q c", r=NC),
                    ag2_in.rearrange("(o q) c -> o q c", o=1).to_broadcast(
                        (NC, NQ, AGC2)))
            else:
                g.collective_compute(
                    "AllGather", OP.bypass,
                    ins=[ag2_in.opt()], outs=[ag2_out.opt()],
                    replica_groups=[list(range(NC))],
                )
            pan2 = constp.tile([128, JT, AGC2], fp32, tag="pan2")
            hx2 = constp.tile([128, JT, 17], bf16, tag="hx2")
            b2 = constp.tile([128, JT], fp32, tag="b2")
            d2 = constp.tile([128, JT], fp32, tag="d2")
            nb2 = constp.tile([128, JT], fp32, tag="nb2")
            b2_bf = constp.tile([128, JT], bf16, tag="b2_bf")
            HJ = JT // 2
            for hf in range(2):
                js = slice(hf * HJ, (hf + 1) * HJ)
                dma.dma_start(
                    pan2[:, js, :],
                    ag2_out[hf * (N // 2):(hf + 1) * (N // 2), :].rearrange(
                        "(t p) c -> p t c", p=128))
                sc.copy(hx2[:, js, :], pan2[:, js, 0:17])
                sc.activation(b2[:, js], pan2[:, js, 17], AF.Exp)
                sc.activation(d2[:, js], pan2[:, js, 17], AF.Exp, scale=ALPHA)
                v.tensor_scalar(nb2[:, js], b2[:, js], -1.0, None, OP.mult)
                v.tensor_copy(b2_bf[:, js], b2[:, js])

            # ---- M. layer-2 attention ----
            acc2 = ps_acc.tile([128, QT, 17], fp32, tag="acc", name="acc2")
            hb2_ps = ps_hb.tile([1, 17], fp32, tag="hb", name="hb2_ps")
            n_act2 = 0
            for jt in range(JT):
                e = _SCHED2[jt]
                pt = ppool.tile([128, NQ], bf16, tag="pt", name="pt2")
                if e == 'A':
                    sc.activation(pt[:], w2b[:], AF.Relu,
                                  bias=nb2[:, jt:jt + 1], scale=d2[:, jt:jt + 1])
                    te.matmul(hb2_ps[:], b2_bf[:, jt:jt + 1], hx2[:, jt, :],
                              start=(n_act2 == 0), stop=(jt == 27))
                    n_act2 += 1
                else:
                    eng = v if e == 'D' else g
                    eng.tensor_scalar(pt[:], w2b[:], d2[:, jt:jt + 1],
                                      b2[:, jt:jt + 1], OP.mult, OP.max)
                for qc in range(QT):
                    te.matmul(acc2[:, qc, :], pt[:, qc * 128:(qc + 1) * 128],
                              hx2[:, jt, :], start=(jt == 0), stop=False)
            hb2_sb = workp.tile([1, 17], bf16, tag="hb2_sb")
            v.tensor_copy(hb2_sb[:], hb2_ps[:])
            for qc in range(QT):
                te.matmul(acc2[:, qc, :], ones1[:], hb2_sb[:],
                          start=False, stop=True)

            # ---- N. normalize, elu, log_softmax, store ----
            r2 = workp.tile([128, QT], fp32, tag="r2")
            v.reciprocal(r2[:], acc2[:, :, 16])
            o_all = workp.tile([128, QT, 16], fp32, tag="o_all")
            for qc in range(QT):
                sc.activation(o_all[:, qc, :], acc2[:, qc, 0:16], AF.Copy,
                              scale=r2[:, qc:qc + 1])
            e2 = workp.tile([128, QT, 16], fp32, tag="e2")
            sc.activation(e2[:].rearrange("p a c -> p (a c)"),
                          o_all[:].rearrange("p a c -> p (a c)"), AF.Exp)
            v.tensor_scalar(e2[:].rearrange("p a c -> p (a c)"),
                            e2[:].rearrange("p a c -> p (a c)"),
                            1.0, 0.0, OP.subtract, OP.min)
            v.tensor_tensor(o_all[:].rearrange("p a c -> p (a c)"),
                            o_all[:].rearrange("p a c -> p (a c)"),
                            e2[:].rearrange("p a c -> p (a c)"), OP.max)
            ee = workp.tile([128, QT, 16], fp32, tag="ee")
            sc.activation(ee[:].rearrange("p a c -> p (a c)"),
                          o_all[:].rearrange("p a c -> p (a c)"), AF.Exp)
            s2s = workp.tile([128, QT], fp32, tag="s2s")
            v.tensor_reduce(s2s[:], ee[:], AX, OP.add)
            lse = workp.tile([128, QT], fp32, tag="lse")
            sc.activation(lse[:], s2s[:], AF.Ln)
            fin = workp.tile([128, QT, 16], fp32, tag="fin")
            for qc in range(QT):
                v.tensor_scalar(fin[:, qc, :], o_all[:, qc, :],
                                lse[:, qc:qc + 1], None, OP.subtract)
            dma.dma_start(out.rearrange("(a p) c -> p a c", p=128), fin[:])

    nc.finalize()
    return nc


def _get_compiled(no_cc=False, no_l1=False):
    key = ("nc", no_cc, no_l1)
    if key not in _CACHE:
        _CACHE[key] = _build_nc(no_cc=no_cc, no_l1=no_l1)
    return _CACHE[key]


def kernel(x, Wh, ah, Wo, ao):
    import ml_dtypes
    from concourse.bass_utils import run_bass_kernel_spmd

    bf = ml_dtypes.bfloat16
    nc = _get_compiled()
    x = np.asarray(x, np.float32)
    Wh = np.asarray(Wh, np.float32)
    ah = np.asarray(ah, np.float32)
    Wo = np.asarray(Wo, np.float32)
    ao = np.asarray(ao, np.float32)

    # host-side relayouts (no math): head-major weight matrix, its transpose,
    # block-diag score matrix, split ao
    Whr = np.ascontiguousarray(
        Wh.transpose(1, 0, 2).reshape(NFEAT, HW))          # [512, 512]
    WhrT = np.ascontiguousarray(Whr.T)
    Asd = np.zeros((HW, 16), np.float32)
    for h in range(NHEADS):
        Asd[h * NHID:(h + 1) * NHID, h] = ah[h, :NHID]      # src
        Asd[h * NHID:(h + 1) * NHID, 8 + h] = ah[h, NHID:]  # dst
    aod = np.stack([ao[:NCLASS], ao[NCLASS:]])              # [2, 16]

    Whr_b = Whr.astype(bf)
    WhrT_b = WhrT.astype(bf)
    Asd_b = Asd.astype(bf)
    Wo_b = np.ascontiguousarray(Wo).astype(bf)

    in_maps = []
    for i in range(NC):
        in_maps.append({
            "xT": np.ascontiguousarray(x[i * NQ:(i + 1) * NQ].T).astype(bf),
            "Whr": Whr_b, "WhrT": WhrT_b, "Asd": Asd_b,
            "Wo": Wo_b, "aod": aod,
        })
    res = run_bass_kernel_spmd(nc, in_maps, list(range(NC)))
    return np.concatenate([res.results[i]["out"] for i in range(NC)], 0)


# revision 33
# speedup vs baseline: 1.0070x; 1.0070x over previous
"""GAT (2-layer, 8-head) fused Bass kernel for 8 trn2 NeuronCores.

Sharding: nodes (rows of x) split 512/core. Per core: h computed key-major
with fused score columns; h (bf16) + s_dst (fp32) AllGather'd; each core
computes its 512xN attention block for all 8 heads; layer-1 output projected
and AllGather'd (18 fp32 cols); each core computes its 512xN layer-2 block
and the final log_softmax rows.

Key algebra: with s_i = h_i . a_src, d_j = h_j . a_dst,
  exp(leakyrelu(s_i + d_j)) = max(exp(s_i)exp(d_j), exp(.2 s_i)exp(.2 d_j))
and softmax over j is invariant to any per-i scale, so the attention
numerator is P[j,i] = max(b_j, w_i * dd_j) with b_j = exp(d_j),
w_i = exp(-0.8 s_i), dd_j = exp(0.2 d_j).

P tiles [128 keys, 512 queries] are produced on three engines:
  DVE/Pool: tensor_scalar (mult, max) -> P
  ACT:      relu(dd_j * w_i - b_j) = P - b_j, single activation op; the
            missing rank-1 term hb[c] = sum_j b_j hx[j,c] over ACT-tiles is
            added back into the PSUM accumulation via two tiny matmuls.
Attention matmuls run with the P chunk [128k x 128q] as the *stationary*
operand and the per-head hx block [128, 65] (64 h cols + ones) as the
*moving* operand: 65 columns/matmul instead of 512 -> ~2x less PE time,
and the output lands query-major so normalize/elu/log_softmax use cheap
per-partition scalars.
"""

import numpy as np

N, NFEAT, NHID, NCLASS, NHEADS = 4096, 512, 64, 16, 8
NC = 8                      # cores
NQ = N // NC                # 512 own nodes per core
QT = NQ // 128              # 4 query tiles per core
JT = N // 128               # 32 key tiles
NCH = JT // NC              # 4 key tiles per AG chunk
ALPHA = 0.2
HW = NHID * NHEADS          # 512
HXC = NHEADS * (NHID + 1)   # 520: per-head 64 h cols + ones col
AGC2 = 18                   # AG2: 16 outh + 1 ones + 1 sdst2

# engine schedule for the 32 P-tiles of each layer-1 head sweep
_ACT1 = (7, 14, 21, 28)
_SCHED1 = ['D'] * JT
for _p in _ACT1:
    _SCHED1[_p] = 'A'
for _p in (2, 9, 23, 30):
    _SCHED1[_p] = 'P'
# layer-2: 32 tiles
_SCHED2 = ['D'] * JT
for _p in (3, 11, 19, 27):
    _SCHED2[_p] = 'A'
for _p in (6, 14, 22, 29):
    _SCHED2[_p] = 'P'

_CACHE = {}


def _build_nc(no_cc=False, no_l1=False):
    import concourse.bass as bass
    import concourse.bacc as bacc
    import concourse.mybir as mybir
    import concourse.tile as tile
    from concourse.masks import make_identity

    fp32 = mybir.dt.float32
    bf16 = mybir.dt.bfloat16
    AX = mybir.AxisListType.X
    OP = mybir.AluOpType
    AF = mybir.ActivationFunctionType

    nc = bacc.Bacc()
    xT = nc.declare_dram_parameter("xT", [NFEAT, NQ], bf16, isOutput=False)
    Whr = nc.declare_dram_parameter("Whr", [NFEAT, HW], bf16, isOutput=False)
    WhrT = nc.declare_dram_parameter("WhrT", [HW, NFEAT], bf16, isOutput=False)
    Asd = nc.declare_dram_parameter("Asd", [HW, 16], bf16, isOutput=False)
    Wo = nc.declare_dram_parameter("Wo", [HW, NCLASS], bf16, isOutput=False)
    aod = nc.declare_dram_parameter("aod", [2, NCLASS], fp32, isOutput=False)
    out = nc.declare_dram_parameter("out", [NQ, NCLASS], fp32, isOutput=True)

    with tile.TileContext(nc) as tc:
        with (
            tc.tile_pool(name="const", bufs=1) as constp,
            tc.tile_pool(name="big", bufs=1) as bigp,
            tc.tile_pool(name="work", bufs=3) as workp,
            tc.tile_pool(name="pp", bufs=56) as ppool,
            tc.tile_pool(name="ps_acc", bufs=3, space="PSUM") as ps_acc,
            tc.tile_pool(name="ps_t", bufs=3, space="PSUM") as ps_t,
            tc.tile_pool(name="ps_hb", bufs=2, space="PSUM") as ps_hb,
            tc.tile_pool(name="dram", bufs=1, space="DRAM") as dramp,
        ):
            v, sc, g, te, dma = nc.vector, nc.scalar, nc.gpsimd, nc.tensor, nc.sync

            ident = constp.tile([128, 128], fp32, tag="ident")
            make_identity(nc, ident[:])
            ident_bf = constp.tile([128, 128], bf16, tag="ident_bf")
            v.tensor_copy(ident_bf[:], ident[:])
            ones1 = constp.tile([1, 128], bf16, tag="ones1")
            g.memset(ones1[:], 1.0)
            # sel[k, h*128+m] = 1 iff k == h (partition-broadcast matmuls)
            self_f = constp.tile([8, 8 * 128], fp32, tag="self_f")
            g.memset(self_f[:], 0.0)
            g.affine_select(
                out=self_f[:].rearrange("k (h m) -> k h m", m=128),
                in_=self_f[:].rearrange("k (h m) -> k h m", m=128),
                compare_op=mybir.AluOpType.not_equal,
                fill=1.0, base=0, channel_multiplier=1,
                pattern=[[-1, 8], [0, 128]])
            sel_bf = constp.tile([8, 8 * 128], bf16, tag="sel_bf")
            sc.copy(sel_bf[:], self_f[:])

            # ---- A. param loads (WhrT/Asd first: the score chain gates the
            # attention panels, which gate all P-tile production) ----
            whrT_sb = constp.tile([128, 4, NFEAT], bf16, tag="whrT_sb")
            dma.dma_start(whrT_sb[:], WhrT.rearrange("(k p) f -> p k f", p=128))
            asd_sb = constp.tile([128, 4, 16], bf16, tag="asd_sb")
            dma.dma_start(asd_sb[:], Asd.rearrange("(k p) s -> p k s", p=128))
            xT_sb = constp.tile([128, 4, NQ], bf16, tag="xT_sb")
            dma.dma_start(xT_sb[:], xT.rearrange("(k p) q -> p k q", p=128))
            whr_sb = constp.tile([128, 4, HW], bf16, tag="whr_sb")
            dma.dma_start(whr_sb[:], Whr.rearrange("(k p) c -> p k c", p=128))
            wo_sb = constp.tile([128, 4, 16], bf16, tag="wo_sb")
            dma.dma_start(wo_sb[:], Wo.rearrange("(k p) s -> p k s", p=128))
            aos_b = constp.tile([128, 16], fp32, tag="aos_b")
            dma.dma_start(aos_b[:], aod[0:1, :].to_broadcast((128, 16)))
            aod_b = constp.tile([128, 16], fp32, tag="aod_b")
            dma.dma_start(aod_b[:], aod[1:2, :].to_broadcast((128, 16)))

            ag1h_in = dramp.tile([NQ, HXC], bf16, tag="ag1h_in")
            ag1h_out = dramp.tile([N, HXC], bf16, tag="ag1h_out",
                                  addr_space="Local" if no_cc else "Shared")
            ag1s_in = dramp.tile([NQ, 8], fp32, tag="ag1s_in")
            ag1s_out = dramp.tile([N, 8], fp32, tag="ag1s_out",
                                  addr_space="Local" if no_cc else "Shared")
            ag2_in = dramp.tile([NQ, AGC2], fp32, tag="ag2_in")
            ag2_out = dramp.tile([N, AGC2], fp32, tag="ag2_out",
                                 addr_space="Local" if no_cc else "Shared")

            # ---- C. Wa_feat = Whr @ Asd; s_own; stage + AllGather s ----
            wa_ps = ps_t.tile([16, NFEAT], fp32, tag="tp", name="wa_ps")
            for k in range(4):
                te.matmul(wa_ps[:], asd_sb[:, k, :], whrT_sb[:, k, :],
                          start=(k == 0), stop=(k == 3))
            waT_sb = constp.tile([16, NFEAT], bf16, tag="waT_sb")
            sc.copy(waT_sb[:], wa_ps[:])
            waf_ps = ps_t.tile([128, 4, 16], bf16, tag="tp", name="waf_ps")
            for k in range(4):
                te.transpose(waf_ps[:, k, :], waT_sb[:, k * 128:(k + 1) * 128],
                             ident_bf[0:16, 0:16])
            wa_f = constp.tile([128, 4, 16], bf16, tag="wa_f")
            v.tensor_copy(wa_f[:], waf_ps[:])

            stgs_all = bigp.tile([128, QT, 16], fp32, tag="stgs_all")
            for qt in range(QT):
                s_ps = ps_t.tile([128, 16], fp32, tag="tp", name="s_ps")
                for k in range(4):
                    te.matmul(s_ps[:], xT_sb[:, k, qt * 128:(qt + 1) * 128],
                              wa_f[:, k, :], start=(k == 0), stop=(k == 3))
                v.tensor_copy(stgs_all[:, qt, :], s_ps[:])
            dma.dma_start(ag1s_in.rearrange("(a p) h -> p a h", p=128),
                          stgs_all[:, :, 8:16])
            if no_cc:
                dma.dma_start(
                    ag1s_out.rearrange("(r q) h -> r q h", r=NC),
                    ag1s_in.rearrange("(o q) h -> o q h", o=1).to_broadcast(
                        (NC, NQ, 8)))
            else:
                g.collective_compute(
                    "AllGather", OP.bypass,
                    ins=[ag1s_in.opt()], outs=[ag1s_out.opt()],
                    replica_groups=[list(range(NC))],
                )
            sd_pan = constp.tile([128, JT * 8], fp32, tag="sd_pan")
            dma.dma_start(
                sd_pan[:].rearrange("p (t h) -> p t h", h=8),
                ag1s_out.rearrange("(t p) h -> p t h", p=128))

            # ---- D. w panel (own s_src): transpose, exp, broadcast ----
            s_fm = ps_t.tile([16, NQ], fp32, tag="tp", name="s_fm")
            for qt in range(QT):
                te.transpose(s_fm[:, qt * 128:(qt + 1) * 128],
                             stgs_all[:, qt, :], ident[0:128, 0:128])
            w_bf = constp.tile([8, NQ], bf16, tag="w_bf")
            sc.activation(w_bf[:], s_fm[0:8, :], AF.Exp, scale=-0.8)
            wb_all = constp.tile([128, NHEADS, NQ], bf16, tag="wb_all")
            for h in range(NHEADS):
                wb_ps = ps_t.tile([128, NQ], fp32, tag="tp", name="wb_ps")
                te.matmul(wb_ps[:], sel_bf[:, h * 128:(h + 1) * 128], w_bf[:],
                          start=True, stop=True)
                sc.copy(wb_all[:, h, :], wb_ps[:])

            # ---- B. h_own key-major; stage + AllGather h ----
            stg = bigp.tile([128, QT, HXC], bf16, tag="stg")
            for qt in range(QT):
                h_ps = ps_acc.tile([128, HW], fp32, tag="acc", name="h_ps")
                for k in range(4):
                    te.matmul(h_ps[:], xT_sb[:, k, qt * 128:(qt + 1) * 128],
                              whr_sb[:, k, :], start=(k == 0), stop=(k == 3))
                eng = sc if qt % 2 else v
                eng_c = eng.copy if qt % 2 else eng.tensor_copy
                eng_c(stg[:, qt, :].rearrange("p (h c) -> p h c", c=65)[:, :, 0:64],
                      h_ps[:].rearrange("p (h c) -> p h c", c=64))
                g.memset(
                    stg[:, qt, :].rearrange("p (h c) -> p h c", c=65)[:, :, 64:65],
                    1.0)
            dma.dma_start(ag1h_in.rearrange("(a p) x -> p a x", p=128), stg[:])

            # ---- E. AG1h replica copies interleaved with hx chunk loads ----
            hx = []
            for c in range(NC):
                if no_cc:
                    dma.dma_start(ag1h_out[c * NQ:(c + 1) * NQ, :],
                                  ag1h_in[:])
                elif c == 0:
                    g.collective_compute(
                        "AllGather", OP.bypass,
                        ins=[ag1h_in.opt()], outs=[ag1h_out.opt()],
                        replica_groups=[list(range(NC))],
                    )
                t = bigp.tile([128, NCH, HXC], bf16, tag=f"hx{c}",
                              name=f"hx{c}")
                dma.dma_start(
                    t[:],
                    ag1h_out[c * NQ:(c + 1) * NQ, :].rearrange(
                        "(a p) x -> p a x", p=128))
                hx.append(t)

            # ---- F. key-side score panels ----
            b_all = constp.tile([128, JT * 8], fp32, tag="b_all")
            sc.activation(b_all[:], sd_pan[:], AF.Exp)
            d_all = constp.tile([128, JT * 8], fp32, tag="d_all")
            sc.activation(d_all[:], sd_pan[:], AF.Exp, scale=ALPHA)
            nb_all = constp.tile([128, JT * 8], fp32, tag="nb_all")
            v.tensor_scalar(nb_all[:], b_all[:], -1.0, None, OP.mult)
            b_bf = constp.tile([128, JT * 8], bf16, tag="b_bf")
            v.tensor_copy(b_bf[:], b_all[:])

            def hx_slice(jt, h, w):
                return hx[jt // NCH][:, jt % NCH, h * 65:h * 65 + w]

            # ---- H. layer-1 attention ----
            xc_pre = [bigp.tile([128, HW], fp32, tag=f"xc{qc}",
                                name=f"xc{qc}") for qc in range(QT)]
            r_pan = constp.tile([128, NHEADS, QT], fp32, tag="r_pan")

            for h in range(NHEADS if not no_l1 else 0):
                acc = ps_acc.tile([128, QT, 65], fp32, tag="acc", name="acc")
                act_jts = []
                hb_ps = ps_hb.tile([1, 65], fp32, tag="hb", name="hb_ps")
                for idx in range(JT):
                    jt = (h * NCH + idx) % JT
                    col = slice(jt * 8 + h, jt * 8 + h + 1)
                    e = _SCHED1[idx]
                    pt = ppool.tile([128, NQ], bf16, tag="pt", name="pt")
                    if e == 'A':
                        sc.activation(pt[:], wb_all[:, h, :], AF.Relu,
                                      bias=nb_all[:, col], scale=d_all[:, col])
                        te.matmul(hb_ps[:], b_bf[:, col], hx_slice(jt, h, 65),
                                  start=(not act_jts), stop=(idx == _ACT1[-1]))
                        act_jts.append(jt)
                    else:
                        eng = v if e == 'D' else g
                        eng.tensor_scalar(pt[:], wb_all[:, h, :],
                                          d_all[:, col], b_all[:, col],
                                          OP.mult, OP.max)
                    for qc in range(QT):
                        te.matmul(acc[:, qc, :],
                                  pt[:, qc * 128:(qc + 1) * 128],
                                  hx_slice(jt, h, 65),
                                  start=(idx == 0), stop=False)
                # rank-1 correction for the ACT-produced tiles: acc += 1 (x) hb
                hb_sb = workp.tile([1, 65], bf16, tag="hb_sb", bufs=2)
                v.tensor_copy(hb_sb[:], hb_ps[:])
                for qc in range(QT):
                    te.matmul(acc[:, qc, :], ones1[:], hb_sb[:],
                              start=False, stop=True)
                # normalize: r = 1/den, xc_pre[:, h*64:] = f * r
                v.reciprocal(r_pan[:, h, :], acc[:, :, 64])
                for qc in range(QT):
                    sc.activation(xc_pre[qc][:, h * 64:(h + 1) * 64],
                                  acc[:, qc, 0:64], AF.Copy,
                                  scale=r_pan[:, h, qc:qc + 1])

            if no_l1:
                for qc in range(QT):
                    g.memset(xc_pre[qc][:], 0.5)

            # ---- I/J. per-query-tile: elu (fp32), transpose, project,
            # stage AG2 ----
            w2tmp = constp.tile([128, QT], fp32, tag="w2tmp")
            stg2 = bigp.tile([128, QT, AGC2], fp32, tag="stg2")
            for qc in range(QT):
                e1 = workp.tile([128, HW], fp32, tag="elu_e", name="e1")
                sc.activation(e1[:], xc_pre[qc][:], AF.Exp)
                g.tensor_scalar(e1[:], e1[:], 1.0, 0.0, OP.subtract, OP.min)
                v.tensor_tensor(xc_pre[qc][:], xc_pre[qc][:], e1[:], OP.max)
                tp = ps_t.tile([128, 4, 128], fp32, tag="tp", name="tp_xc")
                for fc in range(4):
                    te.transpose(tp[:, fc, :],
                                 xc_pre[qc][:, fc * 128:(fc + 1) * 128],
                                 ident[:])
                xcT = bigp.tile([128, 4, 128], bf16, tag=f"xcT{qc}",
                                name=f"xcT{qc}")
                sc.copy(xcT[:], tp[:])
                o_ps = ps_t.tile([128, 16], fp32, tag="tp", name="o_ps")
                for fc in range(4):
                    te.matmul(o_ps[:], xcT[:, fc, :], wo_sb[:, fc, :],
                              start=(fc == 0), stop=(fc == 3))
                v.tensor_copy(stg2[:, qc, 0:16], o_ps[:])
                g.memset(stg2[:, qc, 16:17], 1.0)
                tmp = workp.tile([128, 16], fp32, tag="sdtmp")
                v.scalar_tensor_tensor(tmp[:], o_ps[:], 1.0, aod_b[:],
                                       OP.mult, OP.mult,
                                       accum_out=stg2[:, qc, 17:18])
                tmp2 = workp.tile([128, 16], fp32, tag="sdtmp2")
                v.scalar_tensor_tensor(tmp2[:], o_ps[:], 1.0, aos_b[:],
                                       OP.mult, OP.mult,
                                       accum_out=w2tmp[:, qc:qc + 1])
            dma.dma_start(ag2_in.rearrange("(a p) c -> p a c", p=128), stg2[:])

            # ---- K. w2 broadcast panel ----
            w2e = constp.tile([128, QT], fp32, tag="w2e")
            sc.activation(w2e[:], w2tmp[:], AF.Exp, scale=-0.8)
            w2tp = ps_hb.tile([QT, 128], fp32, tag="hb", name="w2tp")
            te.transpose(w2tp[:], w2e[:], ident[:])
            w2T_bf = constp.tile([QT, 128], bf16, tag="w2T_bf")
            v.tensor_copy(w2T_bf[:], w2tp[:])
            w2b_ps = ps_t.tile([128, QT, 128], fp32, tag="tp", name="w2b_ps")
            for qt in range(QT):
                te.matmul(w2b_ps[:, qt, :],
                          sel_bf[0:QT, qt * 128:(qt + 1) * 128], w2T_bf[:],
                          start=True, stop=True)
            w2b = constp.tile([128, NQ], bf16, tag="w2b")
            sc.copy(w2b[:], w2b_ps[:].rearrange("p a q -> p (a q)"))

            # ---- L. AllGather 2 + panels (two halves for earlier start) ----
            if no_cc:
                dma.dma_start(
                    ag2_out.rearrange("(r q) c -> r q c", r=NC),
                    ag2_in.rearrange("(o q) c -> o q c", o=1).to_broadcast(
                        (NC, NQ, AGC2)))
            else:
                g.collective_compute(
                    "AllGather", OP.bypass,
                    ins=[ag2_in.opt()], outs=[ag2_out.opt()],
                    replica_groups=[list(range(NC))],
                )
            pan2 = constp.tile([128, JT, AGC2], fp32, tag="pan2")
            hx2 = constp.tile([128, JT, 17], bf16, tag="hx2")
            b2 = constp.tile([128, JT], fp32, tag="b2")
            d2 = constp.tile([128, JT], fp32, tag="d2")
            nb2 = constp.tile([128, JT], fp32, tag="nb2")
            b2_bf = constp.tile([128, JT], bf16, tag="b2_bf")
            HJ = JT // 2
            for hf in range(2):
                js = slice(hf * HJ, (hf + 1) * HJ)
                dma.dma_start(
                    pan2[:, js, :],
                    ag2_out[hf * (N // 2):(hf + 1) * (N // 2), :].rearrange(
                        "(t p) c -> p t c", p=128))
                sc.copy(hx2[:, js, :], pan2[:, js, 0:17])
                sc.activation(b2[:, js], pan2[:, js, 17], AF.Exp)
                sc.activation(d2[:, js], pan2[:, js, 17], AF.Exp, scale=ALPHA)
                v.tensor_scalar(nb2[:, js], b2[:, js], -1.0, None, OP.mult)
                v.tensor_copy(b2_bf[:, js], b2[:, js])

            # ---- M. layer-2 attention ----
            acc2 = ps_acc.tile([128, QT, 17], fp32, tag="acc", name="acc2")
            hb2_ps = ps_hb.tile([1, 17], fp32, tag="hb", name="hb2_ps")
            n_act2 = 0
            for jt in range(JT):
                e = _SCHED2[jt]
                pt = ppool.tile([128, NQ], bf16, tag="pt", name="pt2")
                if e == 'A':
                    sc.activation(pt[:], w2b[:], AF.Relu,
                                  bias=nb2[:, jt:jt + 1], scale=d2[:, jt:jt + 1])
                    te.matmul(hb2_ps[:], b2_bf[:, jt:jt + 1], hx2[:, jt, :],
                              start=(n_act2 == 0), stop=(jt == 27))
                    n_act2 += 1
                else:
                    eng = v if e == 'D' else g
                    eng.tensor_scalar(pt[:], w2b[:], d2[:, jt:jt + 1],
                                      b2[:, jt:jt + 1], OP.mult, OP.max)
                for qc in range(QT):
                    te.matmul(acc2[:, qc, :], pt[:, qc * 128:(qc + 1) * 128],
                              hx2[:, jt, :], start=(jt == 0), stop=False)
            hb2_sb = workp.tile([1, 17], bf16, tag="hb2_sb")
            v.tensor_copy(hb2_sb[:], hb2_ps[:])
            for qc in range(QT):
                te.matmul(acc2[:, qc, :], ones1[:], hb2_sb[:],
                          start=False, stop=True)

            # ---- N. normalize, elu, log_softmax, store ----
            r2 = workp.tile([128, QT], fp32, tag="r2")
            v.reciprocal(r2[:], acc2[:, :, 16])
            o_all = workp.tile([128, QT, 16], fp32, tag="o_all")
            for qc in range(QT):
                sc.activation(o_all[:, qc, :], acc2[:, qc, 0:16], AF.Copy,
                              scale=r2[:, qc:qc + 1])
            e2 = workp.tile([128, QT, 16], fp32, tag="e2")
            sc.activation(e2[:].rearrange("p a c -> p (a c)"),
                          o_all[:].rearrange("p a c -> p (a c)"), AF.Exp)
            v.tensor_scalar(e2[:].rearrange("p a c -> p (a c)"),
                            e2[:].rearrange("p a c -> p (a c)"),
                            1.0, 0.0, OP.subtract, OP.min)
            v.tensor_tensor(o_all[:].rearrange("p a c -> p (a c)"),
                            o_all[:].rearrange("p a c -> p (a c)"),
                            e2[:].rearrange("p a c -> p (a c)"), OP.max)
            ee = workp.tile([128, QT, 16], fp32, tag="ee")
            sc.activation(ee[:].rearrange("p a c -> p (a c)"),
                          o_all[:].rearrange("p a c -> p (a c)"), AF.Exp)
            s2s = workp.tile([128, QT], fp32, tag="s2s")
            v.tensor_reduce(s2s[:], ee[:], AX, OP.add)
            lse = workp.tile([128, QT], fp32, tag="lse")
            sc.activation(lse[:], s2s[:], AF.Ln)
            fin = workp.tile([128, QT, 16], fp32, tag="fin")
            for qc in range(QT):
                v.tensor_scalar(fin[:, qc, :], o_all[:, qc, :],
                                lse[:, qc:qc + 1], None, OP.subtract)
            dma.dma_start(out.rearrange("(a p) c -> p a c", p=128), fin[:])

    nc.finalize()
    return nc


def _get_compiled(no_cc=False, no_l1=False):
    key = ("nc", no_cc, no_l1)
    if key not in _CACHE:
        _CACHE[key] = _build_nc(no_cc=no_cc, no_l1=no_l1)
    return _CACHE[key]


def kernel(x, Wh, ah, Wo, ao):
    import ml_dtypes
    from concourse.bass_utils import run_bass_kernel_spmd

    bf = ml_dtypes.bfloat16
    nc = _get_compiled()
    x = np.asarray(x, np.float32)
    Wh = np.asarray(Wh, np.float32)
    ah = np.asarray(ah, np.float32)
    Wo = np.asarray(Wo, np.float32)
    ao = np.asarray(ao, np.float32)

    # host-side relayouts (no math): head-major weight matrix, its transpose,
    # block-diag score matrix, split ao
    Whr = np.ascontiguousarray(
        Wh.transpose(1, 0, 2).reshape(NFEAT, HW))          # [512, 512]
    WhrT = np.ascontiguousarray(Whr.T)
    Asd = np.zeros((HW, 16), np.float32)
    for h in range(NHEADS):
        Asd[h * NHID:(h + 1) * NHID, h] = ah[h, :NHID]      # src
        Asd[h * NHID:(h + 1) * NHID, 8 + h] = ah[h, NHID:]  # dst
    aod = np.stack([ao[:NCLASS], ao[NCLASS:]])              # [2, 16]

    Whr_b = Whr.astype(bf)
    WhrT_b = WhrT.astype(bf)
    Asd_b = Asd.astype(bf)
    Wo_b = np.ascontiguousarray(Wo).astype(bf)

    in_maps = []
    for i in range(NC):
        in_maps.append({
            "xT": np.ascontiguousarray(x[i * NQ:(i + 1) * NQ].T).astype(bf),
            "Whr": Whr_b, "WhrT": WhrT_b, "Asd": Asd_b,
            "Wo": Wo_b, "aod": aod,
        })
    res = run_bass_kernel_spmd(nc, in_maps, list(range(NC)))
    return np.concatenate([res.results[i]["out"] for i in range(NC)], 0)


# revision 34
# speedup vs baseline: 1.0257x; 1.0185x over previous
"""GAT (2-layer, 8-head) fused Bass kernel for 8 trn2 NeuronCores.

Sharding: nodes (rows of x) split 512/core. Per core: h computed key-major
with fused score columns; h (bf16) + s_dst (fp32) AllGather'd; each core
computes its 512xN attention block for all 8 heads; layer-1 output projected
and AllGather'd (18 fp32 cols); each core computes its 512xN layer-2 block
and the final log_softmax rows.

Key algebra: with s_i = h_i . a_src, d_j = h_j . a_dst,
  exp(leakyrelu(s_i + d_j)) = max(exp(s_i)exp(d_j), exp(.2 s_i)exp(.2 d_j))
and softmax over j is invariant to any per-i scale, so the attention
numerator is P[j,i] = max(b_j, w_i * dd_j) with b_j = exp(d_j),
w_i = exp(-0.8 s_i), dd_j = exp(0.2 d_j).

P tiles [128 keys, 512 queries] are produced on three engines:
  DVE/Pool: tensor_scalar (mult, max) -> P
  ACT:      relu(dd_j * w_i - b_j) = P - b_j, single activation op; the
            missing rank-1 term hb[c] = sum_j b_j hx[j,c] over ACT-tiles is
            added back into the PSUM accumulation via two tiny matmuls.
Attention matmuls run with the P chunk [128k x 128q] as the *stationary*
operand and the per-head hx block [128, 65] (64 h cols + ones) as the
*moving* operand: 65 columns/matmul instead of 512 -> ~2x less PE time,
and the output lands query-major so normalize/elu/log_softmax use cheap
per-partition scalars.
"""

import numpy as np

N, NFEAT, NHID, NCLASS, NHEADS = 4096, 512, 64, 16, 8
NC = 8                      # cores
NQ = N // NC                # 512 own nodes per core
QT = NQ // 128              # 4 query tiles per core
JT = N // 128               # 32 key tiles
NCH = JT // NC              # 4 key tiles per AG chunk
ALPHA = 0.2
HW = NHID * NHEADS          # 512
HXC = NHEADS * (NHID + 1)   # 520: per-head 64 h cols + ones col
AGC2 = 18                   # AG2: 16 outh + 1 ones + 1 sdst2

# engine schedule for the 32 P-tiles of each layer-1 head sweep
_ACT1 = (7, 14, 21, 28)
_SCHED1 = ['D'] * JT
for _p in _ACT1:
    _SCHED1[_p] = 'A'
for _p in (2, 9, 23, 30):
    _SCHED1[_p] = 'P'
# layer-2: 32 tiles
_SCHED2 = ['D'] * JT
for _p in (3, 11, 19, 27):
    _SCHED2[_p] = 'A'
for _p in (6, 14, 22, 29):
    _SCHED2[_p] = 'P'

_CACHE = {}


def _build_nc(no_cc=False, no_l1=False):
    import concourse.bass as bass
    import concourse.bacc as bacc
    import concourse.mybir as mybir
    import concourse.tile as tile
    from concourse.masks import make_identity

    fp32 = mybir.dt.float32
    bf16 = mybir.dt.bfloat16
    AX = mybir.AxisListType.X
    OP = mybir.AluOpType
    AF = mybir.ActivationFunctionType

    nc = bacc.Bacc()
    xT = nc.declare_dram_parameter("xT", [NFEAT, NQ], bf16, isOutput=False)
    Whr = nc.declare_dram_parameter("Whr", [NFEAT, HW], bf16, isOutput=False)
    WhrT = nc.declare_dram_parameter("WhrT", [HW, NFEAT], bf16, isOutput=False)
    Asd = nc.declare_dram_parameter("Asd", [HW, 16], bf16, isOutput=False)
    Wo = nc.declare_dram_parameter("Wo", [HW, NCLASS], bf16, isOutput=False)
    aod = nc.declare_dram_parameter("aod", [2, NCLASS], fp32, isOutput=False)
    out = nc.declare_dram_parameter("out", [NQ, NCLASS], fp32, isOutput=True)

    with tile.TileContext(nc) as tc:
        with (
            tc.tile_pool(name="const", bufs=1) as constp,
            tc.tile_pool(name="big", bufs=1) as bigp,
            tc.tile_pool(name="work", bufs=3) as workp,
            tc.tile_pool(name="pp", bufs=56) as ppool,
            tc.tile_pool(name="ps_acc", bufs=3, space="PSUM") as ps_acc,
            tc.tile_pool(name="ps_t", bufs=3, space="PSUM") as ps_t,
            tc.tile_pool(name="ps_hb", bufs=2, space="PSUM") as ps_hb,
            tc.tile_pool(name="dram", bufs=1, space="DRAM") as dramp,
        ):
            v, sc, g, te, dma = nc.vector, nc.scalar, nc.gpsimd, nc.tensor, nc.sync

            ident = constp.tile([128, 128], fp32, tag="ident")
            make_identity(nc, ident[:])
            ident_bf = constp.tile([128, 128], bf16, tag="ident_bf")
            v.tensor_copy(ident_bf[:], ident[:])
            ones1 = constp.tile([1, 128], bf16, tag="ones1")
            g.memset(ones1[:], 1.0)
            # sel[k, h*128+m] = 1 iff k == h (partition-broadcast matmuls)
            self_f = constp.tile([8, 8 * 128], fp32, tag="self_f")
            g.memset(self_f[:], 0.0)
            g.affine_select(
                out=self_f[:].rearrange("k (h m) -> k h m", m=128),
                in_=self_f[:].rearrange("k (h m) -> k h m", m=128),
                compare_op=mybir.AluOpType.not_equal,
                fill=1.0, base=0, channel_multiplier=1,
                pattern=[[-1, 8], [0, 128]])
            sel_bf = constp.tile([8, 8 * 128], bf16, tag="sel_bf")
            sc.copy(sel_bf[:], self_f[:])

            # ---- A. param loads (WhrT/Asd first: the score chain gates the
            # attention panels, which gate all P-tile production) ----
            whrT_sb = constp.tile([128, 4, NFEAT], bf16, tag="whrT_sb")
            dma.dma_start(whrT_sb[:], WhrT.rearrange("(k p) f -> p k f", p=128))
            asd_sb = constp.tile([128, 4, 16], bf16, tag="asd_sb")
            dma.dma_start(asd_sb[:], Asd.rearrange("(k p) s -> p k s", p=128))
            xT_sb = constp.tile([128, 4, NQ], bf16, tag="xT_sb")
            dma.dma_start(xT_sb[:], xT.rearrange("(k p) q -> p k q", p=128))
            whr_sb = constp.tile([128, 4, HW], bf16, tag="whr_sb")
            dma.dma_start(whr_sb[:], Whr.rearrange("(k p) c -> p k c", p=128))
            wo_sb = constp.tile([128, 4, 16], bf16, tag="wo_sb")
            dma.dma_start(wo_sb[:], Wo.rearrange("(k p) s -> p k s", p=128))
            aos_b = constp.tile([128, 16], fp32, tag="aos_b")
            dma.dma_start(aos_b[:], aod[0:1, :].to_broadcast((128, 16)))
            aod_b = constp.tile([128, 16], fp32, tag="aod_b")
            dma.dma_start(aod_b[:], aod[1:2, :].to_broadcast((128, 16)))

            ag1h_in = dramp.tile([NQ, HXC], bf16, tag="ag1h_in")
            ag1h_out = dramp.tile([N, HXC], bf16, tag="ag1h_out",
                                  addr_space="Local" if no_cc else "Shared")
            ag1s_in = dramp.tile([NQ, 8], fp32, tag="ag1s_in")
            ag1s_out = dramp.tile([N, 8], fp32, tag="ag1s_out",
                                  addr_space="Local" if no_cc else "Shared")
            ag2_in = dramp.tile([NQ, AGC2], fp32, tag="ag2_in")
            ag2_out = dramp.tile([N, AGC2], fp32, tag="ag2_out",
                                 addr_space="Local" if no_cc else "Shared")

            # ---- C. Wa_feat = Whr @ Asd; s_own; stage + AllGather s ----
            wa_ps = ps_t.tile([16, NFEAT], fp32, tag="tp", name="wa_ps")
            for k in range(4):
                te.matmul(wa_ps[:], asd_sb[:, k, :], whrT_sb[:, k, :],
                          start=(k == 0), stop=(k == 3))
            waT_sb = constp.tile([16, NFEAT], bf16, tag="waT_sb")
            sc.copy(waT_sb[:], wa_ps[:])
            waf_ps = ps_t.tile([128, 4, 16], bf16, tag="tp", name="waf_ps")
            for k in range(4):
                te.transpose(waf_ps[:, k, :], waT_sb[:, k * 128:(k + 1) * 128],
                             ident_bf[0:16, 0:16])
            wa_f = constp.tile([128, 4, 16], bf16, tag="wa_f")
            v.tensor_copy(wa_f[:], waf_ps[:])

            stgs_all = bigp.tile([128, QT, 16], fp32, tag="stgs_all")
            for qt in range(QT):
                s_ps = ps_t.tile([128, 16], fp32, tag="tp", name="s_ps")
                for k in range(4):
                    te.matmul(s_ps[:], xT_sb[:, k, qt * 128:(qt + 1) * 128],
                              wa_f[:, k, :], start=(k == 0), stop=(k == 3))
                v.tensor_copy(stgs_all[:, qt, :], s_ps[:])
            dma.dma_start(ag1s_in.rearrange("(a p) h -> p a h", p=128),
                          stgs_all[:, :, 8:16])
            if no_cc:
                dma.dma_start(
                    ag1s_out.rearrange("(r q) h -> r q h", r=NC),
                    ag1s_in.rearrange("(o q) h -> o q h", o=1).to_broadcast(
                        (NC, NQ, 8)))
            else:
                g.collective_compute(
                    "AllGather", OP.bypass,
                    ins=[ag1s_in.opt()], outs=[ag1s_out.opt()],
                    replica_groups=[list(range(NC))],
                )
            sd_pan = constp.tile([128, JT * 8], fp32, tag="sd_pan")
            dma.dma_start(
                sd_pan[:].rearrange("p (t h) -> p t h", h=8),
                ag1s_out.rearrange("(t p) h -> p t h", p=128))

            # ---- D. w panel (own s_src): transpose, exp, broadcast ----
            s_fm = ps_t.tile([16, NQ], fp32, tag="tp", name="s_fm")
            for qt in range(QT):
                te.transpose(s_fm[:, qt * 128:(qt + 1) * 128],
                             stgs_all[:, qt, :], ident[0:128, 0:128])
            w_bf = constp.tile([8, NQ], bf16, tag="w_bf")
            sc.activation(w_bf[:], s_fm[0:8, :], AF.Exp, scale=-0.8)
            wb_all = constp.tile([128, NHEADS, NQ], bf16, tag="wb_all")
            for h in range(NHEADS):
                wb_ps = ps_t.tile([128, NQ], fp32, tag="tp", name="wb_ps")
                te.matmul(wb_ps[:], sel_bf[:, h * 128:(h + 1) * 128], w_bf[:],
                          start=True, stop=True)
                sc.copy(wb_all[:, h, :], wb_ps[:])

            # ---- B. h_own key-major; stage + AllGather h ----
            stg = bigp.tile([128, QT, HXC], bf16, tag="stg")
            for qt in range(QT):
                h_ps = ps_acc.tile([128, HW], fp32, tag="acc", name="h_ps")
                for k in range(4):
                    te.matmul(h_ps[:], xT_sb[:, k, qt * 128:(qt + 1) * 128],
                              whr_sb[:, k, :], start=(k == 0), stop=(k == 3))
                eng = sc if qt % 2 else v
                eng_c = eng.copy if qt % 2 else eng.tensor_copy
                eng_c(stg[:, qt, :].rearrange("p (h c) -> p h c", c=65)[:, :, 0:64],
                      h_ps[:].rearrange("p (h c) -> p h c", c=64))
                g.memset(
                    stg[:, qt, :].rearrange("p (h c) -> p h c", c=65)[:, :, 64:65],
                    1.0)
            dma.dma_start(ag1h_in.rearrange("(a p) x -> p a x", p=128), stg[:])

            # ---- E. AG1h replica copies interleaved with hx chunk loads ----
            hx = []
            for c in range(NC):
                if no_cc:
                    dma.dma_start(ag1h_out[c * NQ:(c + 1) * NQ, :],
                                  ag1h_in[:])
                elif c == 0:
                    g.collective_compute(
                        "AllGather", OP.bypass,
                        ins=[ag1h_in.opt()], outs=[ag1h_out.opt()],
                        replica_groups=[list(range(NC))],
                    )
                t = bigp.tile([128, NCH, HXC], bf16, tag=f"hx{c}",
                              name=f"hx{c}")
                dma.dma_start(
                    t[:],
                    ag1h_out[c * NQ:(c + 1) * NQ, :].rearrange(
                        "(a p) x -> p a x", p=128))
                hx.append(t)

            # ---- F. key-side score panels ----
            b_all = constp.tile([128, JT * 8], fp32, tag="b_all")
            sc.activation(b_all[:], sd_pan[:], AF.Exp)
            d_all = constp.tile([128, JT * 8], fp32, tag="d_all")
            sc.activation(d_all[:], sd_pan[:], AF.Exp, scale=ALPHA)
            nb_all = constp.tile([128, JT * 8], fp32, tag="nb_all")
            v.tensor_scalar(nb_all[:], b_all[:], -1.0, None, OP.mult)
            b_bf = constp.tile([128, JT * 8], bf16, tag="b_bf")
            v.tensor_copy(b_bf[:], b_all[:])

            def hx_slice(jt, h, w):
                return hx[jt // NCH][:, jt % NCH, h * 65:h * 65 + w]

            # ---- H. layer-1 attention ----
            xc_pre = [bigp.tile([128, HW], fp32, tag=f"xc{qc}",
                                name=f"xc{qc}") for qc in range(QT)]
            r_pan = constp.tile([128, NHEADS, QT], fp32, tag="r_pan")

            for h in range(NHEADS if not no_l1 else 0):
                acc = ps_acc.tile([128, QT, 65], fp32, tag="acc", name="acc")
                act_jts = []
                hb_ps = ps_hb.tile([1, 65], fp32, tag="hb", name="hb_ps")
                for idx in range(JT):
                    jt = ((h // 2) * NCH + idx) % JT
                    col = slice(jt * 8 + h, jt * 8 + h + 1)
                    e = _SCHED1[idx]
                    pt = ppool.tile([128, NQ], bf16, tag="pt", name="pt")
                    if e == 'A':
                        sc.activation(pt[:], wb_all[:, h, :], AF.Relu,
                                      bias=nb_all[:, col], scale=d_all[:, col])
                        te.matmul(hb_ps[:], b_bf[:, col], hx_slice(jt, h, 65),
                                  start=(not act_jts), stop=(idx == _ACT1[-1]))
                        act_jts.append(jt)
                    else:
                        eng = v if e == 'D' else g
                        eng.tensor_scalar(pt[:], wb_all[:, h, :],
                                          d_all[:, col], b_all[:, col],
                                          OP.mult, OP.max)
                    for qc in range(QT):
                        te.matmul(acc[:, qc, :],
                                  pt[:, qc * 128:(qc + 1) * 128],
                                  hx_slice(jt, h, 65),
                                  start=(idx == 0), stop=False)
                # rank-1 correction for the ACT-produced tiles: acc += 1 (x) hb
                hb_sb = workp.tile([1, 65], bf16, tag="hb_sb", bufs=2)
                v.tensor_copy(hb_sb[:], hb_ps[:])
                for qc in range(QT):
                    te.matmul(acc[:, qc, :], ones1[:], hb_sb[:],
                              start=False, stop=True)
                # normalize: r = 1/den, xc_pre[:, h*64:] = f * r
                v.reciprocal(r_pan[:, h, :], acc[:, :, 64])
                for qc in range(QT):
                    sc.activation(xc_pre[qc][:, h * 64:(h + 1) * 64],
                                  acc[:, qc, 0:64], AF.Copy,
                                  scale=r_pan[:, h, qc:qc + 1])

            if no_l1:
                for qc in range(QT):
                    g.memset(xc_pre[qc][:], 0.5)

            # ---- I/J. per-query-tile: elu (fp32), transpose, project,
            # stage AG2 ----
            w2tmp = constp.tile([128, QT], fp32, tag="w2tmp")
            stg2 = bigp.tile([128, QT, AGC2], fp32, tag="stg2")
            for qc in range(QT):
                e1 = workp.tile([128, HW], fp32, tag="elu_e", name="e1")
                sc.activation(e1[:], xc_pre[qc][:], AF.Exp)
                g.tensor_scalar(e1[:], e1[:], 1.0, 0.0, OP.subtract, OP.min)
                v.tensor_tensor(xc_pre[qc][:], xc_pre[qc][:], e1[:], OP.max)
                tp = ps_t.tile([128, 4, 128], fp32, tag="tp", name="tp_xc")
                for fc in range(4):
                    te.transpose(tp[:, fc, :],
                                 xc_pre[qc][:, fc * 128:(fc + 1) * 128],
                                 ident[:])
                xcT = bigp.tile([128, 4, 128], bf16, tag=f"xcT{qc}",
                                name=f"xcT{qc}")
                sc.copy(xcT[:], tp[:])
                o_ps = ps_t.tile([128, 16], fp32, tag="tp", name="o_ps")
                for fc in range(4):
                    te.matmul(o_ps[:], xcT[:, fc, :], wo_sb[:, fc, :],
                              start=(fc == 0), stop=(fc == 3))
                v.tensor_copy(stg2[:, qc, 0:16], o_ps[:])
                g.memset(stg2[:, qc, 16:17], 1.0)
                tmp = workp.tile([128, 16], fp32, tag="sdtmp")
                v.scalar_tensor_tensor(tmp[:], o_ps[:], 1.0, aod_b[:],
                                       OP.mult, OP.mult,
                                       accum_out=stg2[:, qc, 17:18])
                tmp2 = workp.tile([128, 16], fp32, tag="sdtmp2")
                v.scalar_tensor_tensor(tmp2[:], o_ps[:], 1.0, aos_b[:],
                                       OP.mult, OP.mult,
                                       accum_out=w2tmp[:, qc:qc + 1])
            dma.dma_start(ag2_in.rearrange("(a p) c -> p a c", p=128), stg2[:])

            # ---- K. w2 broadcast panel ----
            w2e = constp.tile([128, QT], fp32, tag="w2e")
            sc.activation(w2e[:], w2tmp[:], AF.Exp, scale=-0.8)
            w2tp = ps_hb.tile([QT, 128], fp32, tag="hb", name="w2tp")
            te.transpose(w2tp[:], w2e[:], ident[:])
            w2T_bf = constp.tile([QT, 128], bf16, tag="w2T_bf")
            v.tensor_copy(w2T_bf[:], w2tp[:])
            w2b_ps = ps_t.tile([128, QT, 128], fp32, tag="tp", name="w2b_ps")
            for qt in range(QT):
                te.matmul(w2b_ps[:, qt, :],
                          sel_bf[0:QT, qt * 128:(qt + 1) * 128], w2T_bf[:],
                          start=True, stop=True)
            w2b = constp.tile([128, NQ], bf16, tag="w2b")
            sc.copy(w2b[:], w2b_ps[:].rearrange("p a q -> p (a q)"))

            # ---- L. AllGather 2 + panels (two halves for earlier start) ----
            if no_cc:
                dma.dma_start(
                    ag2_out.rearrange("(r q) c -> r q c", r=NC),
                    ag2_in.rearrange("(o q) c -> o q c", o=1).to_broadcast(
                        (NC, NQ, AGC2)))
            else:
                g.collective_compute(
                    "AllGather", OP.bypass,
                    ins=[ag2_in.opt()], outs=[ag2_out.opt()],
                    replica_groups=[list(range(NC))],
                )
            pan2 = constp.tile([128, JT, AGC2], fp32, tag="pan2")
            hx2 = constp.tile([128, JT, 17], bf16, tag="hx2")
            b2 = constp.tile([128, JT], fp32, tag="b2")
            d2 = constp.tile([128, JT], fp32, tag="d2")
            nb2 = constp.tile([128, JT], fp32, tag="nb2")
            b2_bf = constp.tile([128, JT], bf16, tag="b2_bf")
            HJ = JT // 2
            for hf in range(2):
                js = slice(hf * HJ, (hf + 1) * HJ)
                dma.dma_start(
                    pan2[:, js, :],
                    ag2_out[hf * (N // 2):(hf + 1) * (N // 2), :].rearrange(
                        "(t p) c -> p t c", p=128))
                sc.copy(hx2[:, js, :], pan2[:, js, 0:17])
                sc.activation(b2[:, js], pan2[:, js, 17], AF.Exp)
                sc.activation(d2[:, js], pan2[:, js, 17], AF.Exp, scale=ALPHA)
                v.tensor_scalar(nb2[:, js], b2[:, js], -1.0, None, OP.mult)
                v.tensor_copy(b2_bf[:, js], b2[:, js])

            # ---- M. layer-2 attention ----
            acc2 = ps_acc.tile([128, QT, 17], fp32, tag="acc", name="acc2")
            hb2_ps = ps_hb.tile([1, 17], fp32, tag="hb", name="hb2_ps")
            n_act2 = 0
            for jt in range(JT):
                e = _SCHED2[jt]
                pt = ppool.tile([128, NQ], bf16, tag="pt", name="pt2")
                if e == 'A':
                    sc.activation(pt[:], w2b[:], AF.Relu,
                                  bias=nb2[:, jt:jt + 1], scale=d2[:, jt:jt + 1])
                    te.matmul(hb2_ps[:], b2_bf[:, jt:jt + 1], hx2[:, jt, :],
                              start=(n_act2 == 0), stop=(jt == 27))
                    n_act2 += 1
                else:
                    eng = v if e == 'D' else g
                    eng.tensor_scalar(pt[:], w2b[:], d2[:, jt:jt + 1],
                                      b2[:, jt:jt + 1], OP.mult, OP.max)
                for qc in range(QT):
                    te.matmul(acc2[:, qc, :], pt[:, qc * 128:(qc + 1) * 128],
                              hx2[:, jt, :], start=(jt == 0), stop=False)
            hb2_sb = workp.tile([1, 17], bf16, tag="hb2_sb")
            v.tensor_copy(hb2_sb[:], hb2_ps[:])
            for qc in range(QT):
                te.matmul(acc2[:, qc, :], ones1[:], hb2_sb[:],
                          start=False, stop=True)

            # ---- N. normalize, elu, log_softmax, store ----
            r2 = workp.tile([128, QT], fp32, tag="r2")
            v.reciprocal(r2[:], acc2[:, :, 16])
            o_all = workp.tile([128, QT, 16], fp32, tag="o_all")
            for qc in range(QT):
                sc.activation(o_all[:, qc, :], acc2[:, qc, 0:16], AF.Copy,
                              scale=r2[:, qc:qc + 1])
            e2 = workp.tile([128, QT, 16], fp32, tag="e2")
            sc.activation(e2[:].rearrange("p a c -> p (a c)"),
                          o_all[:].rearrange("p a c -> p (a c)"), AF.Exp)
            v.tensor_scalar(e2[:].rearrange("p a c -> p (a c)"),
                            e2[:].rearrange("p a c -> p (a c)"),
                            1.0, 0.0, OP.subtract, OP.min)
            v.tensor_tensor(o_all[:].rearrange("p a c -> p (a c)"),
                            o_all[:].rearrange("p a c -> p (a c)"),
                            e2[:].rearrange("p a c -> p (a c)"), OP.max)
            ee = workp.tile([128, QT, 16], fp32, tag="ee")
            sc.activation(ee[:].rearrange("p a c -> p (a c)"),
                          o_all[:].rearrange("p a c -> p (a c)"), AF.Exp)
            s2s = workp.tile([128, QT], fp32, tag="s2s")
            v.tensor_reduce(s2s[:], ee[:], AX, OP.add)
            lse = workp.tile([128, QT], fp32, tag="lse")
            sc.activation(lse[:], s2s[:], AF.Ln)
            fin = workp.tile([128, QT, 16], fp32, tag="fin")
            for qc in range(QT):
                v.tensor_scalar(fin[:, qc, :], o_all[:, qc, :],
                                lse[:, qc:qc + 1], None, OP.subtract)
            dma.dma_start(out.rearrange("(a p) c -> p a c", p=128), fin[:])

    nc.finalize()
    return nc


def _get_compiled(no_cc=False, no_l1=False):
    key = ("nc", no_cc, no_l1)
    if key not in _CACHE:
        _CACHE[key] = _build_nc(no_cc=no_cc, no_l1=no_l1)
    return _CACHE[key]


def kernel(x, Wh, ah, Wo, ao):
    import ml_dtypes
    from concourse.bass_utils import run_bass_kernel_spmd

    bf = ml_dtypes.bfloat16
    nc = _get_compiled()
    x = np.asarray(x, np.float32)
    Wh = np.asarray(Wh, np.float32)
    ah = np.asarray(ah, np.float32)
    Wo = np.asarray(Wo, np.float32)
    ao = np.asarray(ao, np.float32)

    # host-side relayouts (no math): head-major weight matrix, its transpose,
    # block-diag score matrix, split ao
    Whr = np.ascontiguousarray(
        Wh.transpose(1, 0, 2).reshape(NFEAT, HW))          # [512, 512]
    WhrT = np.ascontiguousarray(Whr.T)
    Asd = np.zeros((HW, 16), np.float32)
    for h in range(NHEADS):
        Asd[h * NHID:(h + 1) * NHID, h] = ah[h, :NHID]      # src
        Asd[h * NHID:(h + 1) * NHID, 8 + h] = ah[h, NHID:]  # dst
    aod = np.stack([ao[:NCLASS], ao[NCLASS:]])              # [2, 16]

    Whr_b = Whr.astype(bf)
    WhrT_b = WhrT.astype(bf)
    Asd_b = Asd.astype(bf)
    Wo_b = np.ascontiguousarray(Wo).astype(bf)

    in_maps = []
    for i in range(NC):
        in_maps.append({
            "xT": np.ascontiguousarray(x[i * NQ:(i + 1) * NQ].T).astype(bf),
            "Whr": Whr_b, "WhrT": WhrT_b, "Asd": Asd_b,
            "Wo": Wo_b, "aod": aod,
        })
    res = run_bass_kernel_spmd(nc, in_maps, list(range(NC)))
    return np.concatenate([res.results[i]["out"] for i in range(NC)], 0)
